# revision 1
# baseline (speedup 1.0000x reference)
"""Trainium2 Bass kernel for nn_ConvolutionLayer (FFT conv collapse).

Math: reference computes
    u_fft = rfft(u); ev_fft = rfft(ev)
    p_fft = einsum('bi,kj->bkj', u_fft, ev_fft)      # sums u_fft over i!
    conv  = irfft(p_fft); result = einsum('bkl,k->bl', conv, lam)

The einsum has no shared index, so p_fft[b,k,j] = s_b * ev_fft[k,j] with
s_b = sum_i u_fft[b,i] = u[b,:] @ g   (g = fft(indicator of first L/2+1)).
irfft is R-linear, so with s_b = a_b + i*c_b:
    result[b,:] = a_b * w0 + c_b * w1
    w0 = lam @ ev                       (since irfft(rfft(e)) = e)
    w1 = irfft(i * rfft(w0))            (by linearity over k)
w1 is computed on-device via a 4-step Cooley-Tukey matmul-FFT (64x128),
with the Hermitian symbol (+i / -i / 0) applied in the middle.

Sharding: batch (64) across 8 cores, 8 rows each; the w0/w1 pipeline is
tiny and computed redundantly on every core (no collectives).

Device layouts (per core):
  U    (128p x 512f)   u shard, p = 16*b_loc + t, l = 512*t + f
  EVL  (128p x 2236f)  [EVr | LAMB2]:
        EVr[32s+k, 128t+b]  = ev[k, 128(4t+s)+b]
        LAMB2[32s+k, 60+s]  = LAMB2[32s+k, 124+s] = lam[k]
  xps  (128p x 128f)   [x; x] where x[a,b] = w0[128a+b], built by 16
        accumulating matmuls (lhsT = sliding LAMB2 window)
  FFT: x ->(F64)-> YT ->(*WT)-> ZT ->(F128)-> XT ->(i*sgn)-> X'T
        ->(I128)-> P ->(*Wi)-> Q ->(I64/L, doubled)-> Y2 = [w1; w1]
  final: res_j = a_b * X2 + c_b * Y2 for batch pair b = (2j, 2j+1)
        stacked on the 128 partitions.
"""

import numpy as np

_B, _K, _L = 64, 32, 8192
_NC = 8
_BS = _B // _NC  # 8 batch rows per core
_N1, _N2 = 64, 128  # l = 128*a + b

# ---------------------------------------------------------------- constants


def _build_constants():
    L, N1, N2 = _L, _N1, _N2
    ind = np.zeros(L)
    ind[: L // 2 + 1] = 1.0
    g = np.fft.fft(ind)  # g[n] = sum_{i=0}^{L/2} e^{-2pi i n i/L}

    gU_re = np.tile(g.real.astype(np.float32).reshape(16, 512), (8, 1))
    gU_im = np.tile(g.imag.astype(np.float32).reshape(16, 512), (8, 1))

    # MASK8 (128 x 8): col 2j+i live for partition groups {2j, 2j+1}
    MASK8 = np.zeros((128, 8), np.float32)
    for p in range(128):
        j = (p // 16) // 2
        MASK8[p, 2 * j : 2 * j + 2] = 1.0
    # STK (128 x 128): STK[p, m] = ((p//16) % 2 == m//64)
    STK = np.zeros((128, 128), np.float32)
    for p in range(128):
        STK[p, 64 * ((p // 16) % 2) : 64 * ((p // 16) % 2) + 64] = 1.0

    a_i = np.arange(N1)
    b_i = np.arange(N2)
    F64 = np.exp(-2j * np.pi * np.outer(a_i, a_i) / N1)
    WT = np.exp(-2j * np.pi * np.outer(b_i, a_i) / L)
    F128 = np.exp(-2j * np.pi * np.outer(b_i, b_i) / N2)
    k = a_i[None, :] + N1 * b_i[:, None]  # (128d, 64c)
    sgnT = np.where(
        (k >= 1) & (k <= L // 2 - 1), 1.0, np.where(k > L // 2, -1.0, 0.0)
    )
    I128 = np.exp(+2j * np.pi * np.outer(b_i, b_i) / N2)
    Wi = np.exp(+2j * np.pi * np.outer(a_i, b_i) / L)
    I64s = np.exp(+2j * np.pi * np.outer(a_i, a_i) / N1) / L
    I64re2 = np.hstack([I64s.real, I64s.real])  # (64 x 128) doubled
    I64imN2 = np.hstack([-I64s.imag, -I64s.imag])

    f32 = lambda x: np.ascontiguousarray(np.asarray(x, np.float32))

    p128 = {
        "gU_re": f32(gU_re),
        "gU_im": f32(gU_im),
        "MASK8": f32(MASK8),
        "STK": f32(STK),
        "WT_re": f32(WT.real),
        "WT_im": f32(WT.imag),
        "F128_re": f32(F128.real),
        "F128_im": f32(F128.imag),
        "F128_imN": f32(-F128.imag),
        "sgnT": f32(sgnT),
        "sgnTN": f32(-sgnT),
        "I128_re": f32(I128.real),
        "I128_im": f32(I128.imag),
        "I128_imN": f32(-I128.imag),
    }
    p64 = {
        "F64_re": f32(F64.real),
        "F64_im": f32(F64.imag),
        "Wi_re": f32(Wi.real),
        "Wi_im": f32(Wi.imag),
        "I64_re2": f32(I64re2),
        "I64_imN2": f32(I64imN2),
    }

    def pack(parts):
        off, offs = 0, {}
        for name, arr in parts.items():
            offs[name] = (off, arr.shape[1])
            off += arr.shape[1]
        return np.concatenate(list(parts.values()), axis=1), offs

    CP, cp_off = pack(p128)
    CQ, cq_off = pack(p64)
    return CP, cp_off, CQ, cq_off


_CP, _CP_OFF, _CQ, _CQ_OFF = _build_constants()
_EVL_W = 2048 + 188

# ---------------------------------------------------------------- bass build

_COMPILED = None


def _build_nc():
    import concourse.mybir as mybir
    import concourse.tile as tile
    from concourse import bacc

    f32 = mybir.dt.float32
    Alu = mybir.AluOpType

    nc = bacc.Bacc(None)

    u_d = nc.declare_dram_parameter("u", [128, 512], f32, isOutput=False)
    evl_d = nc.declare_dram_parameter("evl", [128, _EVL_W], f32, isOutput=False)
    cp_d = nc.declare_dram_parameter("cp", list(_CP.shape), f32, isOutput=False)
    cq_d = nc.declare_dram_parameter("cq", list(_CQ.shape), f32, isOutput=False)
    out_d = nc.declare_dram_parameter("out", [_BS, 64, 128], f32, isOutput=True)

    def cp(t, name):
        off, w = _CP_OFF[name]
        return t[:, off : off + w]

    def cq(t, name):
        off, w = _CQ_OFF[name]
        return t[:, off : off + w]

    with tile.TileContext(nc) as tc:
        with (
            tc.tile_pool(name="const", bufs=1) as constp,
            tc.tile_pool(name="sb", bufs=1) as sb,
            tc.tile_pool(name="work", bufs=2) as work,
            tc.tile_pool(name="res", bufs=3) as resp,
            tc.tile_pool(name="psx", bufs=1, space="PSUM") as psx,
            tc.tile_pool(name="psacr", bufs=1, space="PSUM") as psacr,
            tc.tile_pool(name="psfft", bufs=2, space="PSUM") as psfft,
            tc.tile_pool(name="psy", bufs=1, space="PSUM") as psy,
        ):
            EVL = constp.tile([128, _EVL_W], f32)
            U = constp.tile([128, 512], f32)
            CP = constp.tile([128, _CP.shape[1]], f32)
            CQ = constp.tile([64, _CQ.shape[1]], f32)
            nc.sync.dma_start(EVL[:], evl_d[:])
            nc.sync.dma_start(U[:], u_d[:])
            nc.sync.dma_start(CP[:], cp_d[:])
            nc.sync.dma_start(CQ[:], cq_d[:])

            # ---- PE first: xps = [x; x], 16 accumulating matmuls --------
            xps = psx.tile([128, 128], f32)
            for t in range(16):
                nc.tensor.matmul(
                    xps[:],
                    EVL[:, 2048 + 60 - 4 * t : 2048 + 188 - 4 * t],
                    EVL[:, 128 * t : 128 * t + 128],
                    start=(t == 0),
                    stop=(t == 15),
                )

            # ---- DVE: fused per-partition dots  U.g_re, U.g_im ----------
            scratch = sb.tile([128, 512], f32)
            R = sb.tile([128, 2], f32)
            nc.vector.scalar_tensor_tensor(
                scratch[:], U[:], 1.0, cp(CP, "gU_re"),
                op0=Alu.mult, op1=Alu.mult, accum_out=R[:, 0:1],
            )
            nc.vector.scalar_tensor_tensor(
                scratch[:], U[:], 1.0, cp(CP, "gU_im"),
                op0=Alu.mult, op1=Alu.mult, accum_out=R[:, 1:2],
            )
            # R2p (128 x (4j,2i)) = MASK8 * broadcast(R)
            R2p = sb.tile([128, 8], f32)
            nc.vector.tensor_tensor(
                R2p[:].rearrange("p (j i) -> p j i", i=2),
                cp(CP, "MASK8").rearrange("p (j i) -> p j i", i=2),
                R[:].unsqueeze(1).broadcast_to((128, 4, 2)),
                Alu.mult,
            )
            # acrP[m, 2j+i] = ac[2j + m//64, i]  (paired-batch scalars)
            acrP = psacr.tile([128, 8], f32)
            nc.tensor.matmul(acrP[:], cp(CP, "STK"), R2p[:], start=True, stop=True)

            # X2 = [x; x] in SBUF (PE lhsT source + final-stage operand)
            X2 = sb.tile([128, 128], f32)
            nc.vector.tensor_copy(X2[:], xps[:])

            # ---- FFT stage 1: YT[b,c] = sum_a x[a,b] F64[a,c] -----------
            YTre = psfft.tile([128, 64], f32, tag="fftA")
            YTim = psfft.tile([128, 64], f32, tag="fftB")
            nc.tensor.matmul(YTre[:], X2[0:64, :], cq(CQ, "F64_re"), start=True, stop=True)
            nc.tensor.matmul(YTim[:], X2[0:64, :], cq(CQ, "F64_im"), start=True, stop=True)

            # ---- twiddle: ZT = YT * WT (complex) ------------------------
            ZTre = work.tile([128, 64], f32, tag="zt")
            ZTim = work.tile([128, 64], f32, tag="zt2")
            t1 = work.tile([128, 64], f32, tag="t1")
            t2 = work.tile([128, 64], f32, tag="t2")
            nc.vector.tensor_tensor(ZTre[:], YTre[:], cp(CP, "WT_re"), Alu.mult)
            nc.vector.tensor_tensor(t1[:], YTim[:], cp(CP, "WT_im"), Alu.mult)
            nc.vector.tensor_tensor(ZTre[:], ZTre[:], t1[:], Alu.subtract)
            nc.vector.tensor_tensor(ZTim[:], YTre[:], cp(CP, "WT_im"), Alu.mult)
            nc.vector.tensor_tensor(t2[:], YTim[:], cp(CP, "WT_re"), Alu.mult)
            nc.vector.tensor_tensor(ZTim[:], ZTim[:], t2[:], Alu.add)

            # ---- stage 2: XT[d,c] = sum_b F128[b,d] ZT[b,c] -------------
            XTre = psfft.tile([128, 64], f32, tag="fftA")
            XTim = psfft.tile([128, 64], f32, tag="fftB")
            nc.tensor.matmul(XTre[:], cp(CP, "F128_re"), ZTre[:], start=True, stop=False)
            nc.tensor.matmul(XTre[:], cp(CP, "F128_imN"), ZTim[:], start=False, stop=True)
            nc.tensor.matmul(XTim[:], cp(CP, "F128_im"), ZTre[:], start=True, stop=False)
            nc.tensor.matmul(XTim[:], cp(CP, "F128_re"), ZTim[:], start=False, stop=True)

            # ---- symbol: X' = i * sgn * X -------------------------------
            XpTre = work.tile([128, 64], f32, tag="xp")
            XpTim = work.tile([128, 64], f32, tag="xp2")
            nc.vector.tensor_tensor(XpTre[:], XTim[:], cp(CP, "sgnTN"), Alu.mult)
            nc.vector.tensor_tensor(XpTim[:], XTre[:], cp(CP, "sgnT"), Alu.mult)

            # ---- stage 3: P[c,b] = sum_d X'T[d,c] I128[d,b] -------------
            Pre = psfft.tile([64, 128], f32, tag="fftA")
            Pim = psfft.tile([64, 128], f32, tag="fftB")
            nc.tensor.matmul(Pre[:], XpTre[:], cp(CP, "I128_re"), start=True, stop=False)
            nc.tensor.matmul(Pre[:], XpTim[:], cp(CP, "I128_imN"), start=False, stop=True)
            nc.tensor.matmul(Pim[:], XpTre[:], cp(CP, "I128_im"), start=True, stop=False)
            nc.tensor.matmul(Pim[:], XpTim[:], cp(CP, "I128_re"), start=False, stop=True)

            # ---- inverse twiddle: Q = P * Wi (complex) ------------------
            Qre = work.tile([64, 128], f32, tag="q")
            Qim = work.tile([64, 128], f32, tag="q2")
            t3 = work.tile([64, 128], f32, tag="t3")
            t4 = work.tile([64, 128], f32, tag="t4")
            nc.vector.tensor_tensor(Qre[:], Pre[:], cq(CQ, "Wi_re"), Alu.mult)
            nc.vector.tensor_tensor(t3[:], Pim[:], cq(CQ, "Wi_im"), Alu.mult)
            nc.vector.tensor_tensor(Qre[:], Qre[:], t3[:], Alu.subtract)
            nc.vector.tensor_tensor(Qim[:], Pre[:], cq(CQ, "Wi_im"), Alu.mult)
            nc.vector.tensor_tensor(t4[:], Pim[:], cq(CQ, "Wi_re"), Alu.mult)
            nc.vector.tensor_tensor(Qim[:], Qim[:], t4[:], Alu.add)

            # ---- stage 4 (doubled): Y2 = [w1grid; w1grid] ---------------
            Y2 = psy.tile([128, 128], f32)
            nc.tensor.matmul(Y2[:], cq(CQ, "I64_re2"), Qre[:], start=True, stop=False)
            nc.tensor.matmul(Y2[:], cq(CQ, "I64_imN2"), Qim[:], start=False, stop=True)

            # ---- final: paired batches, out DMAs on two queues ----------
            for j in range(_BS // 2):
                tmp = resp.tile([128, 128], f32, tag="tmp")
                nc.vector.tensor_scalar_mul(
                    tmp[:], Y2[:], acrP[:, 2 * j + 1 : 2 * j + 2]
                )
                res = resp.tile([128, 128], f32, tag="res")
                nc.vector.scalar_tensor_tensor(
                    res[:], X2[:], acrP[:, 2 * j : 2 * j + 1], tmp[:],
                    op0=Alu.mult, op1=Alu.add,
                )
                nc.sync.dma_start(out_d[2 * j], res[0:64, :])
                nc.gpsimd.dma_start(out_d[2 * j + 1], res[64:128, :])

    nc.compile()
    return nc


def _get_compiled():
    global _COMPILED
    if _COMPILED is None:
        _COMPILED = _build_nc()
    return _COMPILED


# ---------------------------------------------------------------- entry


def _make_in_maps(u, eigenvectors, eigenvalues):
    u = np.ascontiguousarray(u, np.float32)
    # pure relayout (zero flops): EVr[32s+k, 128t+b] = ev[k, 128(4t+s)+b]
    evr = (
        np.asarray(eigenvectors, np.float32)
        .reshape(_K, 16, 4, 128)
        .transpose(2, 0, 1, 3)
        .reshape(128, 2048)
    )
    lamv = np.asarray(eigenvalues, np.float32)
    lamb2 = np.zeros((128, 188), np.float32)
    for s in range(4):
        lamb2[32 * s : 32 * s + 32, 60 + s] = lamv
        lamb2[32 * s : 32 * s + 32, 124 + s] = lamv
    evl = np.ascontiguousarray(np.hstack([evr, lamb2]))

    in_maps = []
    for c in range(_NC):
        in_maps.append(
            {
                "u": u[c * _BS : (c + 1) * _BS].reshape(128, 512),
                "evl": evl,
                "cp": _CP,
                "cq": _CQ,
            }
        )
    return in_maps, None


def _gather(results):
    outs = [results[c]["out"].reshape(_BS, _L) for c in range(_NC)]
    return np.concatenate(outs, axis=0)


def kernel(u, eigenvectors, eigenvalues):
    from concourse.bass_utils import run_bass_kernel_spmd

    nc = _get_compiled()
    in_maps, _ = _make_in_maps(u, eigenvectors, eigenvalues)
    res = run_bass_kernel_spmd(nc, in_maps, core_ids=list(range(_NC)))
    return _gather(res.results)



# revision 13
# speedup vs baseline: 1.3763x; 1.3763x over previous
"""Trainium2 Bass kernel for nn_ConvolutionLayer (FFT conv collapse), v2.

Math: reference computes
    u_fft = rfft(u); ev_fft = rfft(ev)
    p_fft = einsum('bi,kj->bkj', u_fft, ev_fft)      # sums u_fft over i!
    conv  = irfft(p_fft); result = einsum('bkl,k->bl', conv, lam)

The einsum has no shared index, so p_fft[b,k,j] = s_b * ev_fft[k,j] with
s_b = sum_i u_fft[b,i] = u[b,:] @ g.  irfft is R-linear, so with
s_b = a_b + i*c_b:
    result[b,:] = a_b * V + c_b * W
    V = lam @ ev
    W = irfft(i * rfft(V))

W is computed via a 64x128 Cooley-Tukey matmul-FFT where the middle
(forward-128-stage -> Hermitian symbol i*sgn(k) -> inverse-128-stage)
is collapsed into ONE fixed complex matrix K = F128 @ diag(i*sigma) @ I128
(sigma = +-1 separable part of sgn), plus a rank-2 fixup on spectral row
k2=0 realized as two 1-row matmuls against dense {0,+-2} matrices.

Everything on the PE runs in bf16 (f32 PSUM accumulate); elementwise
twiddles are 3 DVE ops each (two stacked complex products + one stacked
combine using sign-folded constant packs).

Sharding: batch (64) across 8 cores, 8 rows each; the V/W pipeline is
tiny and computed redundantly on every core (no collectives).

Layouts (per core):
  U      blob[:, 0:512]   (128p = 16*b_loc + t) cols [even l | odd l]
  EVT    (128, 2048) bf16: chunk j block m -> grid rows 32(m%2)+4(2j+m//2)+s
  xps    (64, 128) PSUM: x[a, b] = V[128a+b], col-group pairs
  X2     [x; x] (128, 128) bf16
  res    (128, 512) f32: partition 64h+a, free 128j+b -> batch 2j+h
"""

import numpy as np
import ml_dtypes

_B, _K, _L = 64, 32, 8192
_NC = 8
_BS = _B // _NC
_N1, _N2 = 64, 128

_BLOBW = 2504
# blob column offsets (bf16, 128 partitions)
_O_U = 0          # 512
_O_WP = 512       # Wpack 192: [WTre | WTim | WTreN]
_O_KP = 704       # Kpack 384: [KimN | Kre | Kim]
_O_MIMN = 1088    # 128
_O_MIM = 1216     # 128
_O_STK = 1344     # 128
_O_GU = 1472      # 256
_O_M8 = 1728      # 8
_O_P64 = 1736     # 768-wide section on partitions 0:64 (rows 64:128 pad):
                  #   [F64ri 128 | Wipack 384 | I64_2re 128 | I64_2imN 128]

_bf = lambda x: np.asarray(x, ml_dtypes.bfloat16)
_f32 = lambda x: np.ascontiguousarray(np.asarray(x, np.float32))


def _build_const_sections():
    a_i, b_i = np.arange(_N1), np.arange(_N2)
    L = _L
    F64 = np.exp(-2j * np.pi * np.outer(a_i, a_i) / _N1)
    WT = np.exp(-2j * np.pi * np.outer(b_i, a_i) / L)
    F128 = np.exp(-2j * np.pi * np.outer(b_i, b_i) / _N2)
    I128 = np.exp(+2j * np.pi * np.outer(b_i, b_i) / _N2)
    Wi = np.exp(+2j * np.pi * np.outer(a_i, b_i) / L) / L
    I64 = np.exp(+2j * np.pi * np.outer(a_i, a_i) / _N1)
    sigma = np.where(b_i < 64, 1.0, -1.0)
    Kmat = np.einsum('pd,d,dq->pq', F128, 1j * sigma, I128)
    Mim = ((-1.0) ** (b_i[:, None] + b_i[None, :]) - 1.0)

    sec = {}
    sec["Wpack"] = np.hstack([WT.real, WT.imag, -WT.real])
    sec["Kpack"] = np.hstack([-Kmat.imag, Kmat.real, Kmat.imag])
    sec["MimN"] = -Mim
    sec["Mim"] = Mim
    STK = np.zeros((128, 128))
    for p in range(128):
        STK[p, 64 * ((p // 16) % 2): 64 * ((p // 16) % 2) + 64] = 1.0
    sec["STK"] = STK
    ind = np.zeros(L)
    ind[: L // 2 + 1] = 1.0
    g = np.fft.fft(ind)
    sec["GUodd"] = np.tile(g.imag.reshape(16, 512)[:, 1::2], (8, 1))
    M8 = np.zeros((128, 8))
    for p in range(128):
        j = (p // 16) // 2
        M8[p, 2 * j: 2 * j + 2] = 1.0
    sec["MASK8"] = M8
    p64 = np.zeros((128, 768))
    p64[0:64, 0:128] = np.hstack([F64.real, F64.imag])
    p64[0:64, 128:512] = np.hstack([Wi.real, Wi.imag, -Wi.real])
    p64[0:64, 512:640] = np.hstack([I64.real, I64.real])
    p64[0:64, 640:768] = np.hstack([-I64.imag, -I64.imag])
    sec["P64"] = p64

    cmask = np.zeros((128, 1))
    cmask[::16, 0] = L // 2
    return sec, _f32(cmask)


def _build_blob_template():
    sec, cmask = _build_const_sections()
    blob = np.zeros((128, _BLOBW), ml_dtypes.bfloat16)
    blob[:, _O_WP:_O_WP + 192] = _bf(sec["Wpack"])
    blob[:, _O_KP:_O_KP + 384] = _bf(sec["Kpack"])
    blob[:, _O_MIMN:_O_MIMN + 128] = _bf(sec["MimN"])
    blob[:, _O_MIM:_O_MIM + 128] = _bf(sec["Mim"])
    blob[:, _O_STK:_O_STK + 128] = _bf(sec["STK"])
    blob[:, _O_GU:_O_GU + 256] = _bf(sec["GUodd"])
    blob[:, _O_M8:_O_M8 + 8] = _bf(sec["MASK8"])
    blob[:, _O_P64:_O_P64 + 768] = _bf(sec["P64"])
    return blob, cmask


_BLOB_T, _CMASK = _build_blob_template()

# ---------------------------------------------------------------- bass build

_COMPILED = None


def _build_nc():
    import concourse.mybir as mybir
    import concourse.tile as tile
    from concourse import bacc

    f32 = mybir.dt.float32
    bf16 = mybir.dt.bfloat16
    Alu = mybir.AluOpType
    Act = mybir.ActivationFunctionType

    nc = bacc.Bacc(None)

    lamw_d = nc.declare_dram_parameter("lamw", [128, 60], bf16, isOutput=False)
    evt_d = nc.declare_dram_parameter("evt", [128, 2048], bf16, isOutput=False)
    blob_d = nc.declare_dram_parameter("blob", [128, _BLOBW], bf16, isOutput=False)
    aux_d = nc.declare_dram_parameter("aux", [128, 1], f32, isOutput=False)
    out_d = nc.declare_dram_parameter("out", [2, 128, 256], f32, isOutput=True)

    with tile.TileContext(nc) as tc:
        with (
            tc.tile_pool(name="sb", bufs=1) as sb,
            tc.tile_pool(name="ps", bufs=1, space="PSUM") as ps,
        ):
            LAMW = sb.tile([128, 60], bf16)
            BLOB = sb.tile([128, _BLOBW], bf16)
            AUX = sb.tile([128, 1], f32)
            EVT = [
                sb.tile([128, 512], bf16, name=f"evt{j}", tag=f"evt{j}")
                for j in range(4)
            ]

            # ---- input DMAs: spread across HWDGE issuers + SWDGE -------
            nc.scalar.dma_start(LAMW[:], lamw_d[:])
            nc.sync.dma_start(EVT[0][:], evt_d[:, 0:512])
            nc.scalar.dma_start(EVT[1][:], evt_d[:, 512:1024])
            nc.sync.dma_start(EVT[2][:], evt_d[:, 1024:1536])
            nc.scalar.dma_start(EVT[3][:], evt_d[:, 1536:2048])
            nc.gpsimd.dma_start(BLOB[:], blob_d[:])
            nc.gpsimd.dma_start(AUX[:], aux_d[:])

            # ---- V grid doubled [x; x]: 32 matmuls, 4 col-group streams
            xps = ps.tile([128, 128], f32)
            for j in range(4):
                for m in range(4):
                    g, i = m % 2, 2 * j + m // 2
                    for h in range(2):
                        nc.tensor.matmul(
                            xps[64 * h + 32 * g: 64 * h + 32 * g + 32, :],
                            LAMW[:, 28 - 4 * i: 60 - 4 * i],
                            EVT[j][:, 128 * m: 128 * m + 128],
                            start=(i == 0),
                            stop=(i == 7),
                            skip_group_check=True,
                            tile_position=(0, 64 * h + 32 * g),
                        )

            # ---- U dots (DVE) + acr broadcast (PE) ---------------------
            R0 = sb.tile([128, 1], f32)
            R = sb.tile([128, 2], f32)
            dsc = sb.tile([128, 256], bf16)
            nc.vector.tensor_reduce(
                R0[:], BLOB[:, _O_U:_O_U + 256], axis=mybir.AxisListType.X,
                op=Alu.add,
            )
            nc.vector.scalar_tensor_tensor(
                R[:, 0:1], BLOB[:, _O_U:_O_U + 1], AUX[:, 0:1], R0[:],
                op0=Alu.mult, op1=Alu.add,
            )
            nc.vector.scalar_tensor_tensor(
                dsc[:], BLOB[:, _O_U + 256:_O_U + 512], 1.0,
                BLOB[:, _O_GU:_O_GU + 256],
                op0=Alu.mult, op1=Alu.mult, accum_out=R[:, 1:2],
            )
            R2p = sb.tile([128, 8], bf16)
            nc.vector.tensor_tensor(
                R2p[:].rearrange("p (j i) -> p j i", i=2),
                BLOB[:, _O_M8:_O_M8 + 8].rearrange("p (j i) -> p j i", i=2),
                R[:].unsqueeze(1).broadcast_to((128, 4, 2)),
                Alu.mult,
            )
            acrPps = ps.tile([128, 8], f32)
            nc.tensor.matmul(
                acrPps[:], BLOB[:, _O_STK:_O_STK + 128], R2p[:],
                start=True, stop=True,
            )
            acrS = sb.tile([128, 8], f32)
            nc.scalar.copy(acrS[:], acrPps[:])

            # ---- X2 = [x; x] (already doubled in PSUM) -----------------
            X2 = sb.tile([128, 128], bf16)
            nc.vector.tensor_copy(X2[:], xps[:])

            # ---- Xa_j = acr[:, 2j] * X2 (ACT, off critical path) -------
            Xa = sb.tile([128, 512], bf16)
            for j in range(4):
                nc.scalar.activation(
                    Xa[:, 128 * j: 128 * j + 128], X2[:], Act.Copy,
                    bias=0.0, scale=acrS[:, 2 * j: 2 * j + 1],
                )

            # ---- stage1: Yps = x.T @ [F64re | F64im] -------------------
            Yps = ps.tile([128, 128], f32)
            nc.tensor.matmul(
                Yps[:], X2[0:64, :], BLOB[0:64, _O_P64:_O_P64 + 128],
                start=True, stop=True,
            )

            # ---- twiddle1 (3 DVE ops) ----------------------------------
            S1t = sb.tile([128, 256], bf16)
            ZT = sb.tile([128, 128], bf16)
            nc.vector.tensor_tensor(
                S1t[:, 0:128], Yps[:], BLOB[:, _O_WP:_O_WP + 128], Alu.mult,
            )
            nc.vector.tensor_tensor(
                S1t[:, 128:256], Yps[:], BLOB[:, _O_WP + 64:_O_WP + 192],
                Alu.mult,
            )
            s1v = S1t[:].rearrange("p (i f) -> p i f", i=2)
            nc.vector.tensor_tensor(
                ZT[:].rearrange("p (i f) -> p i f", i=2),
                s1v[:, :, 0:64], s1v[:, :, 64:128], Alu.subtract,
            )

            # ---- K stage: Pps = Zre.T@[Kre|Kim] + Zim.T@[KimN|Kre] -----
            Pps = ps.tile([64, 256], f32)
            nc.tensor.matmul(
                Pps[:], ZT[:, 0:64], BLOB[:, _O_KP + 128:_O_KP + 384],
                start=True, stop=False,
            )
            nc.tensor.matmul(
                Pps[:], ZT[:, 64:128], BLOB[:, _O_KP:_O_KP + 256],
                start=False, stop=True,
            )
            nc.tensor.matmul(
                Pps[0:1, 0:128], ZT[:, 64:65], BLOB[:, _O_MIMN:_O_MIMN + 128],
                start=False, stop=True, skip_group_check=True,
            )
            nc.tensor.matmul(
                Pps[0:1, 128:256], ZT[:, 0:1], BLOB[:, _O_MIM:_O_MIM + 128],
                start=False, stop=True, skip_group_check=True,
            )

            # ---- twiddle2 (3 DVE ops) ----------------------------------
            T2t = sb.tile([64, 512], bf16)
            D = sb.tile([64, 256], bf16)
            nc.vector.tensor_tensor(
                T2t[:, 0:256], Pps[:],
                BLOB[0:64, _O_P64 + 128:_O_P64 + 384], Alu.mult,
            )
            nc.vector.tensor_tensor(
                T2t[:, 256:512], Pps[:],
                BLOB[0:64, _O_P64 + 256:_O_P64 + 512], Alu.mult,
            )
            t2v = T2t[:].rearrange("p (i f) -> p i f", i=2)
            nc.vector.tensor_tensor(
                D[:].rearrange("p (i f) -> p i f", i=2),
                t2v[:, :, 0:128], t2v[:, :, 128:256], Alu.subtract,
            )

            # ---- stage4: Y2 = I64_2re.T @ Dre + I64_2imN.T @ Dim -------
            Y2ps = ps.tile([128, 128], f32)
            nc.tensor.matmul(
                Y2ps[:], BLOB[0:64, _O_P64 + 512:_O_P64 + 640], D[:, 0:128],
                start=True, stop=False,
            )
            nc.tensor.matmul(
                Y2ps[:], BLOB[0:64, _O_P64 + 640:_O_P64 + 768], D[:, 128:256],
                start=False, stop=True,
            )

            # ---- final: res[:, 128j:] = Y2*c_j + Xa_j ------------------
            res01 = sb.tile([128, 256], f32)
            res23 = sb.tile([128, 256], f32)
            rtiles = {0: res01, 1: res01, 2: res23, 3: res23}
            for j in range(4):
                rt = rtiles[j]
                off = 128 * (j % 2)
                nc.vector.scalar_tensor_tensor(
                    rt[:, off:off + 128], Y2ps[:],
                    acrS[:, 2 * j + 1: 2 * j + 2],
                    Xa[:, 128 * j: 128 * j + 128],
                    op0=Alu.mult, op1=Alu.add,
                )

            # ---- output DMAs (plain; batch un-permute happens on host) -
            nc.scalar.dma_start(out_d[0], res01[:])
            nc.sync.dma_start(out_d[1], res23[:])

    nc.compile()
    return nc


def _get_compiled():
    global _COMPILED
    if _COMPILED is None:
        _COMPILED = _build_nc()
    return _COMPILED


# ---------------------------------------------------------------- entry


def _build_evt(ev):
    # chunk j block m holds (g, i) = (m % 2, 2j + m // 2):
    # EVT[32s+k, 512j + 128m + b] = ev[k, 128*(32g + 4i + s) + b]
    e = np.asarray(ev, np.float32).reshape(_K, 2, 8, 4, 128)  # [k,g,i,s,b]
    # evt[s, k, j, m, b] = e[k, m%2, 2j + m//2, s, b]
    em = e.reshape(_K, 2, 4, 2, 4, 128)                # [k, g, jj, i2, s, b]
    # i = 2*jj + i2 with g from m%2: m = 2*i2 + g ... build via transpose:
    # target index [s, k, j(=jj), m(=2*i2+g), b]
    evt = em.transpose(4, 0, 2, 3, 1, 5).reshape(128, 4, 4, 128)
    # now dims [32s+k? no: (s,k) merged=128, jj, (i2,g), b] -> m = 2*i2+g
    evt = evt.reshape(128, 2048)
    return _bf(evt)


def _build_lamw(lam):
    lamw = np.zeros((128, 60), np.float32)
    for s in range(4):
        lamw[32 * s: 32 * s + 32, 28 + s] = lam
    return _bf(lamw)


def _make_in_maps(u, eigenvectors, eigenvalues):
    evt = _build_evt(eigenvectors)
    lamw = _build_lamw(np.asarray(eigenvalues, np.float32))
    in_maps = []
    for c in range(_NC):
        blob = _BLOB_T.copy()
        U = np.asarray(u[c * _BS:(c + 1) * _BS], np.float32).reshape(128, 512)
        blob[:, 0:256] = _bf(U[:, 0::2])
        blob[:, 256:512] = _bf(U[:, 1::2])
        in_maps.append({
            "lamw": lamw,
            "evt": evt,
            "blob": blob,
            "aux": _CMASK,
        })
    return in_maps, None


def _gather(results):
    out = np.zeros((_B, _L), np.float32)
    for c in range(_NC):
        r = np.asarray(results[c]["out"], np.float32)   # (2, 128, 256)
        full = np.concatenate([r[0], r[1]], axis=1)     # (128, 512)
        # partition p = 64h + a; free = 128j + b; batch 2j+h; l = 128a+b
        v = full.reshape(2, 64, 4, 128).transpose(2, 0, 1, 3)
        out[c * _BS:(c + 1) * _BS] = v.reshape(_BS, _L)
    return out


def kernel(u, eigenvectors, eigenvalues):
    from concourse.bass_utils import run_bass_kernel_spmd

    nc = _get_compiled()
    in_maps, _ = _make_in_maps(u, eigenvectors, eigenvalues)
    res = run_bass_kernel_spmd(nc, in_maps, core_ids=list(range(_NC)))
    return _gather(res.results)
